# revision 3
# baseline (speedup 1.0000x reference)
"""ChameleonAttention on 8 Trainium2 NeuronCores.

Tensor-parallel over heads: each core owns 4 of the 32 heads.
  - Wq/Wk/Wv sharded column-wise (512 cols/core), Wo row-wise (512 rows/core)
  - per-head LayerNorm + RoPE computed on-chip, gamma/beta replicated
  - causal attention with block-skipping (only lower-triangular key tiles)
  - per-core partial output [S, HID] summed on host (the TP all-reduce)

Precision: projections contract K=4096 in float32r (full PE rate, ~1e-4 rel
err). Attention operands + output projection in fp16 (values are O(1) after
LayerNorm / softmax; keeps QT/KT/V resident in SBUF with no DRAM spills).
Softmax uses exp(s*scale - 4) with no running max (LayerNormed q/k bound the
logits), denominator via an all-ones stationary matmul, division deferred to
after the P@V accumulation.

Projection runs in two S-halves so the fp32 accumulators + hidden-state
panels fit in SBUF; PSUM accumulates each 512-wide K-panel, SBUF fp32
accumulators carry the full K=4096 contraction.
"""
import math
from contextlib import ExitStack

import numpy as np

_S = 2048
_HID = 4096
_D = 128
_NC = 8
_CPW = _HID // _NC  # columns per core (512) = 4 heads
_HPC = _CPW // _D  # heads per core (4)
_KP = 512  # contraction panel (4 k-tiles of 128)
_NPAN = _HID // _KP  # 8 panels
_ROPE_THETA = 10000.0
_EPS = 1e-5
_EXP_BIAS = -4.0

_cache = {}


def _build(S):
    import concourse.tile as tile
    from concourse import bacc, mybir
    from concourse.masks import make_identity

    f32 = mybir.dt.float32
    f32r = mybir.dt.float32r
    f16 = mybir.dt.float16
    mul = mybir.AluOpType.mult
    add = mybir.AluOpType.add

    NM = S // 128  # s-tiles (16)
    NQB = S // 512  # query banks (4)
    NMH = NM // 2  # s-tiles per half (8)
    SH = S // 2  # rows per half
    KPT = _KP // 128  # k-tiles per panel (4)

    nc = bacc.Bacc("TRN2", target_bir_lowering=False, debug=False)

    hT_d = nc.dram_tensor("hT", [_HID, S], f32r, kind="ExternalInput")
    w_d = {
        t: nc.dram_tensor(f"w{t}", [_HID, _CPW], f32r, kind="ExternalInput")
        for t in ("q", "k", "v")
    }
    wo_d = nc.dram_tensor("wo", [_CPW, _HID], f16, kind="ExternalInput")
    cos_d = nc.dram_tensor("cos", [S, _D], f32, kind="ExternalInput")
    sin_d = nc.dram_tensor("sin", [S, _D], f32, kind="ExternalInput")
    vec_d = {}
    for t in ("q", "k"):
        for nm in ("g", "sg", "b", "sb"):
            vec_d[nm + t] = nc.dram_tensor(
                f"{nm}{t}", [1, _D], f32, kind="ExternalInput"
            )
    masks_d = nc.dram_tensor("masks", [4, 128, 512], f16, kind="ExternalInput")
    out_d = nc.dram_tensor("out", [S, _HID], f16, kind="ExternalOutput")

    with tile.TileContext(nc) as tc, ExitStack() as ctx:
        # ---- persistent small constants ----
        persist = ctx.enter_context(tc.tile_pool(name="persist", bufs=1))
        ident = persist.tile([128, 128], f32)
        make_identity(nc, ident[:])
        ones16 = persist.tile([128, 128], f16)
        nc.vector.memset(ones16[:], 1.0)
        ebias = persist.tile([128, 1], f32)
        nc.vector.memset(ebias[:], _EXP_BIAS)
        epst = persist.tile([128, 1], f32)
        nc.vector.memset(epst[:], _EPS)
        bc = {}
        for nm, d in vec_d.items():
            t = persist.tile([128, _D], f32, name=f"bc_{nm}")
            nc.gpsimd.dma_start(out=t[:], in_=d.ap().to_broadcast((128, _D)))
            bc[nm] = t

        # ---- fp16 attention operands, filled by phase P ----
        att = ctx.enter_context(tc.tile_pool(name="att", bufs=1))
        qt_t = [att.tile([128, S], f16, name=f"qt{h}") for h in range(_HPC)]
        kt_t = [att.tile([128, S], f16, name=f"kt{h}") for h in range(_HPC)]
        v16 = att.tile([128, NM, 512], f16)

        # ================= phase P: QKV projection =================
        with ExitStack() as pctx:
            acc_pool = pctx.enter_context(tc.tile_pool(name="acc", bufs=1))
            acc = {
                t: acc_pool.tile([128, NMH, 512], f32, name=f"acc_{t}")
                for t in ("q", "k", "v")
            }
            proj = pctx.enter_context(tc.tile_pool(name="proj", bufs=6))
            wpool = pctx.enter_context(tc.tile_pool(name="wpool", bufs=14))
            lntmp = pctx.enter_context(tc.tile_pool(name="lntmp", bufs=3))
            pps = pctx.enter_context(tc.tile_pool(name="pps", bufs=4, space="PSUM"))
            tps = pctx.enter_context(tc.tile_pool(name="tps", bufs=2, space="PSUM"))

            for half in range(2):
                for kp in range(_NPAN):
                    hts = []
                    for k4 in range(KPT):
                        ht = proj.tile([128, SH], f32r, tag="ht",
                                       name=f"ht_{half}_{kp}_{k4}")
                        kk = kp * KPT + k4
                        nc.sync.dma_start(
                            ht[:],
                            hT_d[kk * 128 : (kk + 1) * 128,
                                 half * SH : (half + 1) * SH],
                        )
                        hts.append(ht)
                    wts = {}
                    for t in ("q", "k", "v"):
                        wts[t] = []
                        for k4 in range(KPT):
                            wt = wpool.tile([128, 512], f32r, tag="w",
                                            name=f"w{t}_{half}_{kp}_{k4}")
                            kk = kp * KPT + k4
                            nc.sync.dma_start(
                                wt[:], w_d[t][kk * 128 : (kk + 1) * 128, :]
                            )
                            wts[t].append(wt)
                    for t in ("q", "k", "v"):
                        for m in range(NMH):
                            ps = pps.tile([128, 512], f32, tag="ps",
                                          name=f"ps_{half}_{kp}_{t}_{m}")
                            for k4 in range(KPT):
                                nc.tensor.matmul(
                                    ps[:],
                                    hts[k4][:, m * 128 : (m + 1) * 128],
                                    wts[t][k4][:],
                                    start=(k4 == 0),
                                    stop=(k4 == KPT - 1),
                                )
                            if kp == 0:
                                nc.vector.tensor_copy(acc[t][:, m, :], ps[:])
                            else:
                                nc.vector.tensor_tensor(
                                    acc[t][:, m, :], acc[t][:, m, :], ps[:], op=add
                                )

                # ---- epilogue for this half: LN + RoPE + transpose, cast V ----
                for m in range(NMH):
                    gm = half * NMH + m  # global s-tile
                    nc.vector.tensor_copy(v16[:, gm, :], acc["v"][:, m, :])
                for t in ("q", "k"):
                    dst = qt_t if t == "q" else kt_t
                    for m in range(NMH):
                        gm = half * NMH + m
                        cs = lntmp.tile([128, _D], f32, tag="cs", name=f"cs_{half}{t}{m}")
                        sn = lntmp.tile([128, _D], f32, tag="sn", name=f"sn_{half}{t}{m}")
                        nc.sync.dma_start(cs[:], cos_d[gm * 128 : (gm + 1) * 128, :])
                        nc.sync.dma_start(sn[:], sin_d[gm * 128 : (gm + 1) * 128, :])
                        c1 = lntmp.tile([128, _D], f32, tag="c1", name=f"c1_{half}{t}{m}")
                        dd = lntmp.tile([128, _D], f32, tag="dd", name=f"dd_{half}{t}{m}")
                        ee = lntmp.tile([128, _D], f32, tag="ee", name=f"ee_{half}{t}{m}")
                        t2 = lntmp.tile([128, _D], f32, tag="t2", name=f"e2_{half}{t}{m}")
                        nc.vector.tensor_tensor(c1[:], cs[:], bc["g" + t][:], op=mul)
                        nc.vector.tensor_tensor(dd[:], sn[:], bc["sg" + t][:], op=mul)
                        nc.vector.tensor_tensor(ee[:], cs[:], bc["b" + t][:], op=mul)
                        nc.vector.tensor_tensor(t2[:], sn[:], bc["sb" + t][:], op=mul)
                        nc.vector.tensor_tensor(ee[:], ee[:], t2[:], op=add)
                        for h in range(_HPC):
                            x = acc[t][:, m, h * _D : (h + 1) * _D]
                            st = lntmp.tile([128, 6], f32, tag="st",
                                            name=f"st_{half}{t}{m}{h}")
                            mv = lntmp.tile([128, 2], f32, tag="mv",
                                            name=f"mv_{half}{t}{m}{h}")
                            nc.vector.bn_stats(out=st[:], in_=x)
                            nc.vector.bn_aggr(out=mv[:], in_=st[:])
                            rstd = lntmp.tile([128, 1], f32, tag="rs",
                                              name=f"rs_{half}{t}{m}{h}")
                            nc.scalar.activation(
                                out=rstd[:], in_=mv[:, 1:2],
                                func=mybir.ActivationFunctionType.Sqrt,
                                bias=epst[:], scale=1.0,
                            )
                            nc.vector.reciprocal(out=rstd[:], in_=rstd[:])
                            xn = lntmp.tile([128, _D], f32, tag="xn",
                                            name=f"xn_{half}{t}{m}{h}")
                            nc.vector.tensor_scalar(
                                out=xn[:], in0=x, scalar1=mv[:, 0:1], scalar2=rstd[:],
                                op0=mybir.AluOpType.subtract, op1=mul,
                            )
                            xp = lntmp.tile([128, _D], f32, tag="xp",
                                            name=f"xp_{half}{t}{m}{h}")
                            half_d = _D // 2
                            nc.vector.tensor_copy(xp[:, :half_d], xn[:, half_d:])
                            nc.vector.tensor_copy(xp[:, half_d:], xn[:, :half_d])
                            q1 = lntmp.tile([128, _D], f32, tag="q1",
                                            name=f"q1_{half}{t}{m}{h}")
                            q2 = lntmp.tile([128, _D], f32, tag="q2",
                                            name=f"q2_{half}{t}{m}{h}")
                            nc.vector.tensor_tensor(q1[:], xn[:], c1[:], op=mul)
                            nc.vector.tensor_tensor(q2[:], xp[:], dd[:], op=mul)
                            nc.vector.tensor_tensor(q1[:], q1[:], q2[:], op=add)
                            nc.vector.tensor_tensor(q1[:], q1[:], ee[:], op=add)
                            tp = tps.tile([128, _D], f32, tag="tp",
                                          name=f"tp_{half}{t}{m}{h}")
                            nc.tensor.transpose(tp[:], q1[:], ident[:])
                            nc.vector.tensor_copy(
                                dst[h][:, gm * 128 : (gm + 1) * 128], tp[:]
                            )

        # ============ phases A+O share at_t ============
        with ExitStack() as aoctx:
            aop = aoctx.enter_context(tc.tile_pool(name="aop", bufs=1))
            at_t = [aop.tile([128, S], f16, name=f"at{h}") for h in range(_HPC)]

            # ---------- phase A: causal attention ----------
            with ExitStack() as actx:
                mpool = actx.enter_context(tc.tile_pool(name="mpool", bufs=1))
                upool = actx.enter_context(tc.tile_pool(name="upool", bufs=4))
                rpool = actx.enter_context(tc.tile_pool(name="rpool", bufs=2))
                sps = actx.enter_context(tc.tile_pool(name="sps", bufs=3, space="PSUM"))
                ops = actx.enter_context(tc.tile_pool(name="ops", bufs=2, space="PSUM"))
                dps = actx.enter_context(tc.tile_pool(name="dps", bufs=2, space="PSUM"))

                mask_t = mpool.tile([128, 4, 512], f16)
                nc.sync.dma_start(mask_t[:], masks_d.ap().rearrange("t p n -> p t n"))

                scale = 1.0 / math.sqrt(_D)
                for h in range(_HPC):
                    for qb in range(NQB):
                        o_ps = ops.tile([128, 512], f32, tag="o", name=f"o_{h}_{qb}")
                        d_ps = dps.tile([128, 512], f32, tag="d", name=f"d_{h}_{qb}")
                        nkt = 4 * qb + 4
                        for kt in range(nkt):
                            s_ps = sps.tile([128, 512], f32, tag="s",
                                            name=f"s_{h}_{qb}_{kt}")
                            nc.tensor.matmul(
                                s_ps[:],
                                kt_t[h][:, kt * 128 : (kt + 1) * 128],
                                qt_t[h][:, qb * 512 : (qb + 1) * 512],
                                start=True, stop=True,
                            )
                            u = upool.tile([128, 512], f16, tag="u",
                                           name=f"u_{h}_{qb}_{kt}")
                            nc.scalar.activation(
                                out=u[:], in_=s_ps[:],
                                func=mybir.ActivationFunctionType.Exp,
                                bias=ebias[:], scale=scale,
                            )
                            toff = kt - 4 * qb
                            if toff >= 0:
                                nc.vector.tensor_tensor(
                                    u[:], u[:], mask_t[:, toff, :], op=mul
                                )
                            nc.tensor.matmul(
                                o_ps[:], v16[:, kt, h * _D : (h + 1) * _D], u[:],
                                start=(kt == 0), stop=(kt == nkt - 1),
                            )
                            nc.tensor.matmul(
                                d_ps[:], ones16[:], u[:],
                                start=(kt == 0), stop=(kt == nkt - 1),
                            )
                        rec = rpool.tile([128, 512], f32, tag="r", name=f"r_{h}_{qb}")
                        nc.vector.reciprocal(out=rec[:], in_=d_ps[:])
                        nc.vector.tensor_tensor(
                            at_t[h][:, qb * 512 : (qb + 1) * 512], o_ps[:], rec[:],
                            op=mul,
                        )

            # ---------- phase O: output projection ----------
            with ExitStack() as octx:
                wop = octx.enter_context(tc.tile_pool(name="wop", bufs=8))
                outp = octx.enter_context(tc.tile_pool(name="outp", bufs=3))
                xps = octx.enter_context(tc.tile_pool(name="xps", bufs=3, space="PSUM"))

                for n in range(_HID // 512):
                    wos = []
                    for k4 in range(_HPC):
                        wo_t = wop.tile([128, 512], f16, tag="wo", name=f"wo_{n}_{k4}")
                        nc.sync.dma_start(
                            wo_t[:],
                            wo_d[k4 * 128 : (k4 + 1) * 128, n * 512 : (n + 1) * 512],
                        )
                        wos.append(wo_t)
                    for m in range(NM):
                        ps = xps.tile([128, 512], f32, tag="x", name=f"x_{n}_{m}")
                        for k4 in range(_HPC):
                            nc.tensor.matmul(
                                ps[:],
                                at_t[k4][:, m * 128 : (m + 1) * 128],
                                wos[k4][:],
                                start=(k4 == 0), stop=(k4 == _HPC - 1),
                            )
                        ot = outp.tile([128, 512], f16, tag="ot", name=f"ot_{n}_{m}")
                        nc.vector.tensor_copy(ot[:], ps[:])
                        nc.sync.dma_start(
                            out_d[m * 128 : (m + 1) * 128, n * 512 : (n + 1) * 512],
                            ot[:],
                        )

    nc.compile()
    return nc


def _host_prep(hidden_states, position_ids, Wq, Wk, Wv, Wo, qn_w, qn_b, kn_w, kn_b):
    S = hidden_states.shape[1]
    hT = np.ascontiguousarray(np.asarray(hidden_states, np.float32)[0].T)
    pos = np.asarray(position_ids, np.float32)[0]  # [S]
    inv = 1.0 / (_ROPE_THETA ** (np.arange(0, _D, 2, dtype=np.float32) / _D))
    fr = pos[:, None] * inv[None, :]  # [S, D/2]
    emb = np.concatenate([fr, fr], axis=1)  # [S, D]
    cos = np.cos(emb).astype(np.float32)
    sin = np.sin(emb).astype(np.float32)

    half = _D // 2
    perm = np.concatenate([np.arange(half, _D), np.arange(0, half)])
    sign = np.concatenate([-np.ones(half, np.float32), np.ones(half, np.float32)])

    def vecs(g, b):
        g = np.asarray(g, np.float32).reshape(_D)
        b = np.asarray(b, np.float32).reshape(_D)
        return (
            g.reshape(1, _D).copy(),
            (sign * g[perm]).reshape(1, _D),
            b.reshape(1, _D).copy(),
            (sign * b[perm]).reshape(1, _D),
        )

    gq, sgq, bq, sbq = vecs(qn_w, qn_b)
    gk, sgk, bk, sbk = vecs(kn_w, kn_b)

    masks = np.zeros((4, 128, 512), np.float16)
    for t in range(4):
        kk = np.arange(128)[:, None] + t * 128
        qq = np.arange(512)[None, :]
        masks[t] = (kk <= qq).astype(np.float16)

    common = {
        "hT": hT,
        "cos": cos,
        "sin": sin,
        "gq": gq, "sgq": sgq, "bq": bq, "sbq": sbq,
        "gk": gk, "sgk": sgk, "bk": bk, "sbk": sbk,
        "masks": masks,
    }
    Wq = np.asarray(Wq, np.float32)
    Wk = np.asarray(Wk, np.float32)
    Wv = np.asarray(Wv, np.float32)
    Wo16 = np.asarray(Wo, np.float32).astype(np.float16)
    in_maps = []
    for c in range(_NC):
        sl = slice(c * _CPW, (c + 1) * _CPW)
        m = dict(common)
        m["wq"] = np.ascontiguousarray(Wq[:, sl])
        m["wk"] = np.ascontiguousarray(Wk[:, sl])
        m["wv"] = np.ascontiguousarray(Wv[:, sl])
        m["wo"] = np.ascontiguousarray(Wo16[sl, :])
        in_maps.append(m)
    return in_maps


def kernel(**inputs) -> np.ndarray:
    from concourse.bass_utils import run_bass_kernel_spmd

    hidden_states = np.asarray(inputs["hidden_states"])
    S = hidden_states.shape[1]
    if S not in _cache:
        _cache[S] = _build(S)
    nc = _cache[S]

    in_maps = _host_prep(
        hidden_states,
        inputs["position_ids"],
        inputs["Wq"], inputs["Wk"], inputs["Wv"], inputs["Wo"],
        inputs["qn_w"], inputs["qn_b"], inputs["kn_w"], inputs["kn_b"],
    )
    res = run_bass_kernel_spmd(nc, in_maps, list(range(_NC)))
    out = np.zeros((S, _HID), np.float32)
    for c in range(_NC):
        out += res.results[c]["out"].astype(np.float32)
    return out.reshape(1, S, _HID)
